# revision 19
# baseline (speedup 1.0000x reference)
"""Self-contained TRN2 Bass kernel for nn_MultiHeadAttentionLayer
(GNN multi-head attention message passing), 8 NeuronCores.

kernel(**inputs) takes the FULL unsharded inputs (h, Wq, bq, Wk, bk, Wv,
bv, src, dst) as numpy arrays and returns the FULL [N, H, D] float32
output. Sharding: edges are partitioned by dst range across the 8 cores
(no collectives needed); each core projects K/V for all nodes into two
src-half tables, gathers K|V rows per edge with dma_gather round-robined
over SWDGE queues 1-3 (async descriptor generation on separate Q7 core
pairs), computes scores/softmax weights on DVE/ACT, and segment-sums
into per-superblock PSUM accumulators via one-hot (fp8) matmuls on the
TensorEngine. The per-chunk back half (exp, V-weighting, scatter) is
software-pipelined one chunk behind the front half so no engine queue
blocks on a cross-engine dependency.
"""

from dataclasses import dataclass, field

import numpy as np
import ml_dtypes

import concourse.bass as bass
import concourse.tile as tile
from concourse import bacc, mybir
from concourse.bass import ts
from concourse.bass_utils import run_bass_kernel_spmd

BF16 = ml_dtypes.bfloat16


def _register_mul_cumsum():
    """Custom DVE op: out = inclusive prefix sum of (Src0*Src1) over the
    free-dim stream. Segmented dot products then fall out of differences
    of segment-end columns."""
    from concourse import dve_ops as DV
    from concourse.dve_spec import Scan, Src0, Src1

    NAME = "MUL_CUMSUM_ANT"
    for op in DV.OPS:
        if op.name == NAME:
            return op
    spec = DV.Spec(
        body=Scan(op=DV.AluOp.ADD, expr=(Src0 * Src1)),
        reference=lambda in0, in1, s0, s1: (in0 * in1).cumsum(axis=-1),
    )
    from concourse.dve_ops import lower, has_src1, DveOpSpec, DveOp
    row = max(DV._SUB_OPCODE_FOR_NAME.values()) + 1
    assert row < 0x20
    shas = {}
    for ver in ("v3", "v4"):
        tmp = DveOpSpec(name=NAME, opcode=row, uops=lower(spec, ver=ver),
                        rd1_en=has_src1(spec))
        shas[ver] = tmp.sha(ver)
    op = DveOp(NAME, spec, subdim=False, uops_sha=shas)
    DV.OPS.append(op)
    DV.CUSTOM_DVE_SPECS[NAME] = spec
    DV._SUB_OPCODE_FOR_NAME[NAME] = row
    return op
F32 = np.float32
AF = mybir.ActivationFunctionType
ALU = mybir.AluOpType

CH_CAP = 12          # max blocks per gather/compute chunk
NGRP = 2             # src groups (table halves)
N_SWDGE_Q = 4        # SWDGE queues allocated (queue 0 unused for gathers:
                     # it is synchronous on the Pool engine; 1-3 are async)
QW = 2               # blocks per Q-gather PSUM piece
WB = 4               # phase-A table-write batch (chunks per DMA)


@dataclass
class Cfg:
    N: int
    IN: int
    H: int
    D: int
    n_cores: int = 8
    NPC: int = 0
    NT: int = 0
    NSB: int = 0
    CHG: list = field(default_factory=list)   # [j][g] blocks
    ASSIGN: list = field(default_factory=list)  # [core][pos] -> global sb

    @property
    def C(self):
        return self.H * self.D

    @property
    def KA(self):
        return self.IN // 128

    @property
    def GBOUND(self):
        return [0, self.NT // 4, self.NT]

    @property
    def NTGS(self):
        gb = self.GBOUND
        return [gb[g + 1] - gb[g] for g in range(NGRP)]

    @property
    def SBLK(self):
        return [sum(row) for row in self.CHG]


def make_cfg(N, IN, H, D, src, dst, n_cores=8):
    cfg = Cfg(N=N, IN=IN, H=H, D=D, n_cores=n_cores)
    cfg.NPC = -(-N // (n_cores * 128)) * 128
    cfg.NT = cfg.NPC * n_cores
    cfg.NSB = cfg.NPC // 128
    src = np.asarray(src)
    dst = np.asarray(dst)
    gsb = dst // 128                       # global super-block of each edge
    nsb_tot = cfg.NSB * n_cores
    grp = (src >= cfg.GBOUND[1]).astype(np.int64)
    counts = np.zeros((nsb_tot, NGRP), dtype=np.int64)
    np.add.at(counts, (np.minimum(gsb, nsb_tot - 1), grp), 1)
    order = np.argsort(-counts.sum(axis=1), kind="stable")
    cfg.ASSIGN = [[int(order[k * n_cores + i]) for k in range(cfg.NSB)]
                  for i in range(n_cores)]
    cfg.CHG = []
    for k in range(cfg.NSB):
        g_sbs = order[k * n_cores:(k + 1) * n_cores]
        cfg.CHG.append([max(1, int(-(-counts[g_sbs, g].max() // 128)))
                        for g in range(NGRP)])
    return cfg


def chunk_plan(cfg):
    """[(j, g, cb, CH, key)] in canonical (j, g) order."""
    plan = []
    key = 0
    for j in range(cfg.NSB):
        for g in range(NGRP):
            CHG = cfg.CHG[j][g]
            cap = CH_CAP if not (g == NGRP - 1 and j == cfg.NSB - 1) \
                else max(4, -(-CHG // 4))
            nparts = -(-CHG // cap)
            base = -(-CHG // nparts)
            b0 = 0
            while b0 < CHG:
                plan.append((j, g, b0, min(base, CHG - b0), key))
                key += 1
                b0 += base
    return plan


def _wrap16(idx, epb):
    base = idx.reshape(epb // 16, 16).T.astype(np.int16)
    return np.tile(base, (8, 1))


def prep(cfg: Cfg, h, Wq, bq, Wk, bk, Wv, bv, src, dst):
    N, IN, H, D, C = cfg.N, cfg.IN, cfg.H, cfg.D, cfg.C
    scale = 1.0 / np.sqrt(np.float32(D))

    hT = np.zeros((IN, cfg.NT), dtype=BF16)
    hT[:, :N] = np.asarray(h).T.astype(BF16)
    wkv = np.concatenate([np.asarray(Wk), np.asarray(Wv)], axis=1).astype(BF16)
    bkv = np.concatenate([np.asarray(bk), np.asarray(bv)])[None, :].astype(BF16)
    wq = (np.asarray(Wq) * scale).astype(BF16)
    bqs = (np.asarray(bq) * scale)[None, :].astype(BF16)

    src = np.asarray(src).astype(np.int64)
    dst = np.asarray(dst).astype(np.int64)

    sum_blk = sum(cfg.SBLK)
    sum_epb = sum_blk * 128

    gsb_of = dst // 128
    grp_of = (src >= cfg.GBOUND[1]).astype(np.int64)
    in_maps = []
    for i in range(cfg.n_cores):
        srcidx = np.zeros(sum_epb, dtype=np.int64)
        ld = np.full((sum_blk, 128), 255, dtype=np.int64)
        off_e = 0
        off_b = 0
        for j in range(cfg.NSB):
            g_sb = cfg.ASSIGN[i][j]
            insb = gsb_of == g_sb
            es, ed, eg = src[insb], dst[insb] - g_sb * 128, grp_of[insb]
            for g in range(NGRP):
                chg = cfg.CHG[j][g]
                gsel = eg == g
                gidx = es[gsel] - cfg.GBOUND[g]
                cnt = gidx.shape[0]
                epb = chg * 128
                assert cnt <= epb, (i, j, g, cnt, epb)
                srcidx[off_e:off_e + cnt] = gidx
                ldj = np.full(epb, 255, dtype=np.int64)
                ldj[:cnt] = ed[gsel]
                ld[off_b:off_b + chg, :] = ldj.reshape(chg, 128)
                off_e += epb
                off_b += chg

        srcw_parts = []
        off = 0
        for j in range(cfg.NSB):
            for g in range(NGRP):
                epb = cfg.CHG[j][g] * 128
                srcw_parts.append(_wrap16(srcidx[off:off + epb], epb))
                off += epb
        srcw = np.concatenate(srcw_parts, axis=1)

        # one-hot dst matrices in fp8 (0/1 exact): halves their DMA vs bf16
        marange = np.arange(128, dtype=np.int64)
        onehot = (ld[:, :, None] == marange[None, None, :])       # [bb, e, m]
        FP8 = mybir.dt.np(mybir.dt.float8e4)
        Sh = np.ascontiguousarray(onehot.transpose(1, 0, 2)).astype(FP8)
        ShT = np.ascontiguousarray(onehot.transpose(2, 0, 1)).astype(FP8)

        cols = np.concatenate(
            [np.arange(cfg.ASSIGN[i][j] * 128, cfg.ASSIGN[i][j] * 128 + 128)
             for j in range(cfg.NSB)])
        in_maps.append({
            "hT": hT,
            "hTq": np.ascontiguousarray(hT[:, cols]),
            "wkv": wkv, "bkv": bkv, "wq": wq, "bq": bqs,
            "srcidx": srcw,
            "Sh": Sh, "ShT": ShT,
        })
    return in_maps


def build(cfg: Cfg):
    MUL_CUMSUM = _register_mul_cumsum()
    N, IN, H, D, C = cfg.N, cfg.IN, cfg.H, cfg.D, cfg.C
    KA = cfg.KA
    C2 = 2 * C
    CZ = C + H
    sum_blk = sum(cfg.SBLK)
    sum_epb = sum_blk * 128
    bf = mybir.dt.bfloat16
    f32 = mybir.dt.float32
    fp8 = mybir.dt.float8e4

    nc = bacc.Bacc("TRN2", target_bir_lowering=False, debug=False,
                   num_swdge_queues=N_SWDGE_Q)
    hT = nc.dram_tensor("hT", [IN, cfg.NT], bf, kind="ExternalInput").ap()
    hTq = nc.dram_tensor("hTq", [IN, cfg.NPC], bf, kind="ExternalInput").ap()
    wkv = nc.dram_tensor("wkv", [IN, C2], bf, kind="ExternalInput").ap()
    bkv = nc.dram_tensor("bkv", [1, C2], bf, kind="ExternalInput").ap()
    wq = nc.dram_tensor("wq", [IN, C], bf, kind="ExternalInput").ap()
    bq = nc.dram_tensor("bq", [1, C], bf, kind="ExternalInput").ap()
    srcidx = nc.dram_tensor("srcidx", [128, sum_epb // 16], mybir.dt.int16,
                            kind="ExternalInput").ap()
    Sh_d = nc.dram_tensor("Sh", [128, sum_blk, 128], fp8,
                          kind="ExternalInput").ap()
    ShT_d = nc.dram_tensor("ShT", [128, sum_blk, 128], fp8,
                           kind="ExternalInput").ap()
    out = nc.dram_tensor("out", [cfg.NPC, C], f32, kind="ExternalOutput").ap()

    NCG0 = cfg.NTGS[0] // 128
    NCALL = cfg.NT // 128

    with tile.TileContext(nc) as tc:
        with (
            tc.tile_pool(name="dram", bufs=1, space="DRAM") as dramp,
            tc.tile_pool(name="const", bufs=1) as constp,
            tc.tile_pool(name="pa_h", bufs=1) as pah,
            tc.tile_pool(name="pa_ps", bufs=2, space="PSUM") as paps,
            tc.tile_pool(name="pa_sb", bufs=2) as pasb,
            tc.tile_pool(name="pb_g", bufs=5) as pg,
            tc.tile_pool(name="pb_t", bufs=3) as pt,
            tc.tile_pool(name="pb_c", bufs=2) as pc,
            tc.tile_pool(name="pb_w", bufs=2) as pw,
            tc.tile_pool(name="pb_s", bufs=2) as psm,
            tc.tile_pool(name="pb_ps", bufs=3, space="PSUM") as pps,
            tc.tile_pool(name="pb_qps", bufs=2, space="PSUM") as pqps,
        ):
            kv_t = [dramp.tile([cfg.NTGS[g], C2], bf, name=f"kv_t{g}")
                    for g in range(NGRP)]

            wkvt = constp.tile([128, KA, C2], bf)
            nc.sync.dma_start(wkvt[:], wkv.rearrange("(a p) c -> p a c", p=128))
            wqt = constp.tile([128, KA, C], bf)
            nc.sync.dma_start(wqt[:], wq.rearrange("(a p) c -> p a c", p=128))
            bkvt = constp.tile([1, C2], bf)
            nc.sync.dma_start(bkvt[:], bkv[:])
            bqt = constp.tile([1, C], bf)
            nc.sync.dma_start(bqt[:], bq[:])
            ones1 = constp.tile([1, 128], bf)
            nc.vector.memset(ones1[:], 1.0)
            srct = constp.tile([128, sum_epb // 16], mybir.dt.int16)
            nc.sync.dma_start(srct[:], srcidx[:])
            qs = constp.tile([128, cfg.NSB, C], bf)
            bias_v = constp.tile([128, C], f32)
            bias_q = constp.tile([128, C], bf)
            acc = [constp.tile([128, CZ], f32, name=f"acc{j}")
                   for j in range(cfg.NSB)]

            hts = pah.tile([128, KA, cfg.NT], bf)
            hT_r = hT.rearrange("(a p) n -> p a n", p=128)
            htq = pah.tile([128, KA, cfg.NPC], bf)
            nc.sync.dma_start(htq[:], hTq.rearrange("(a p) n -> p a n", p=128))
            NSPL = 8
            SPL = cfg.NT // NSPL
            for sp in range(NSPL):
                nc.sync.dma_start(hts[:, :, ts(sp, SPL)],
                                  hT_r[:, :, ts(sp, SPL)])

            # ---------------- phase A ----------------
            bps = paps.tile([128, C2], f32, tag="psA", name="bps")
            nc.tensor.matmul(out=bps[:], lhsT=ones1[:], rhs=bkvt[:],
                             start=True, stop=True)
            nc.vector.tensor_copy(bias_v[:], bps[:, C:C2])
            bpq = paps.tile([128, C], f32, tag="psA", name="bpq")
            nc.tensor.matmul(out=bpq[:], lhsT=ones1[:], rhs=bqt[:],
                             start=True, stop=True)
            nc.vector.tensor_copy(bias_q[:], bpq[:])

            awbuf = [None]

            def emit_A(cc):
                if cc % WB == 0:
                    awbuf[0] = pasb.tile([128, WB, C2], bf, tag="bufA",
                                         name=f"wbuf{cc}")
                ps = paps.tile([128, C2], f32, tag="psA")
                for a in range(KA):
                    nc.tensor.matmul(out=ps[:], lhsT=hts[:, a, ts(cc, 128)],
                                     rhs=wkvt[:, a, :], start=(a == 0),
                                     stop=(a == KA - 1))
                sl = cc % WB
                # K half raw (bk cancels per-dst), V raw (bv folded into
                # the finalize as (wV + bv*z)/z)
                nc.scalar.copy(awbuf[0][:, sl, :], ps[:])
                if sl == WB - 1:
                    cc0 = cc - (WB - 1)
                    g = 0 if cc < NCG0 else 1
                    b0 = cc0 - (0 if g == 0 else NCG0)
                    nc.sync.dma_start(
                        kv_t[g].rearrange("(b p) c -> p b c", p=128)
                        [:, b0:b0 + WB, :],
                        awbuf[0][:])

            for cc in range(NCG0):
                emit_A(cc)

            # Q projection right after the group-0 table
            for qc in range(cfg.NSB):
                psq = paps.tile([128, C], f32, tag="psA", name="psq")
                for a in range(KA):
                    nc.tensor.matmul(out=psq[:], lhsT=htq[:, a, ts(qc, 128)],
                                     rhs=wqt[:, a, :], start=(a == 0),
                                     stop=(a == KA - 1))
                nc.vector.tensor_tensor(qs[:, qc, :], psq[:], bias_q[:],
                                        op=ALU.add)

            # ---------------- phase B ----------------
            grp_off = {}
            off_b = 0
            for j in range(cfg.NSB):
                for g in range(NGRP):
                    grp_off[(j, g)] = off_b
                    off_b += cfg.CHG[j][g]

            pswz_of = {}
            gq_counter = [0]
            plan_all = chunk_plan(cfg)
            pending = []          # software-pipelined back halves
            pending_fin = []      # deferred finalizes

            def back_half(st):
                (j, g, sc, kvg, sh, wvz, CH, flags) = st
                nc.scalar.activation(wvz[:, :, C:CZ], sc[:], AF.Exp)
                nc.vector.tensor_tensor(
                    wvz[:, :, 0:C].rearrange("p b (h d) -> p b h d", d=D),
                    kvg[:, :, C:C2].rearrange("p b (h d) -> p b h d", d=D),
                    wvz[:, :, C:CZ].unsqueeze(3)
                    .broadcast_to([128, CH, H, D]),
                    op=ALU.mult)
                first, last = flags
                if g == 0:
                    # group 0: rotating PSUM tile, banked into the
                    # per-superblock SBUF accumulator
                    psz = pps.tile([128, CZ], f32, tag="pswz")
                    for b in range(CH):
                        nc.tensor.matmul(
                            out=psz[:], lhsT=sh[:, b, :], rhs=wvz[:, b, :],
                            start=(b == 0), stop=(b == CH - 1))
                    if first:
                        nc.scalar.copy(acc[j][:], psz[:])
                    else:
                        nc.vector.tensor_tensor(acc[j][:], acc[j][:],
                                                psz[:], op=ALU.add)
                else:
                    # group 1: whole superblock contribution stays in PSUM
                    if first:
                        pswz_of[j] = pps.tile([128, CZ], f32, tag="pswz",
                                              name=f"pswz{j}")
                    pswz = pswz_of[j]
                    for b in range(CH):
                        nc.tensor.matmul(
                            out=pswz[:], lhsT=sh[:, b, :], rhs=wvz[:, b, :],
                            start=(first and b == 0),
                            stop=(last and b == CH - 1))
                    if last:
                        pending_fin.append(j)

            def drain_one(q):
                if q:
                    back_half(q.pop(0))

            def do_finalize():
                if pending_fin:
                    finalize(pending_fin.pop(0))

            def process_group(j, g, tbl, last_of_sb):
                gb = grp_off[(j, g)]
                chunks = [(cb, CH, key) for (jj, gg, cb, CH, key) in plan_all
                          if jj == j and gg == g]
                for (ci, (cb, CH, key)) in enumerate(chunks):
                    cbk = gb + cb
                    ce = cbk * 128
                    EPC = CH * 128
                    kvg = pg.tile([128, CH, C2], bf, tag="kvg")
                    nc.gpsimd.dma_gather(
                        kvg[:], tbl, srct[:, ce // 16:(ce + EPC) // 16],
                        EPC, EPC, C2, single_packet=False,
                        queue_num=1 + gq_counter[0] % 3)
                    gq_counter[0] += 1

                    sh = pt.tile([128, CH, 128], fp8, tag="sh")
                    nc.sync.dma_start(sh[:], Sh_d[:, cbk:cbk + CH, :])
                    sht = pt.tile([128, CH, 128], fp8, tag="sht")
                    nc.sync.dma_start(sht[:], ShT_d[:, cbk:cbk + CH, :])

                    # --- Q gather (PE) + PSUM->SBUF copy on ACT ---
                    qg = pc.tile([128, CH, C], bf, tag="qg")
                    for b0q in range(0, CH, QW):
                        bw = min(QW, CH - b0q)
                        qps = pqps.tile([128, QW, C], f32, tag="qps")
                        for b in range(b0q, b0q + bw):
                            nc.tensor.matmul(out=qps[:, b - b0q, :],
                                             lhsT=sht[:, b, :],
                                             rhs=qs[:, j, :],
                                             start=True, stop=True)
                        nc.scalar.copy(qg[:, b0q:b0q + bw, :],
                                       qps[:, 0:bw, :])

                    # --- score: cumsum of K*Qg, then segment-end
                    # differences ---
                    NSEG = CH * H
                    cum = pc.tile([128, CH * C + 1], f32, tag="cum")
                    nc.vector.memset(cum[:, 0:1], 0.0)
                    nc.vector._custom_dve(
                        MUL_CUMSUM, out=cum[:, 1:1 + CH * C],
                        in0=kvg[:, :, 0:C], in1=qg[:])
                    sc = pc.tile([128, CH, H], f32, tag="sc")
                    nc.vector.tensor_tensor(
                        sc[:].rearrange("p b h -> p (b h)"),
                        cum[:, D:CH * C + 1:D],
                        cum[:, 0:NSEG * D:D],
                        op=ALU.subtract)

                    wvz = pw.tile([128, CH, CZ], bf, tag="wvz")
                    pending.append((j, g, sc, kvg, sh, wvz, CH,
                                    (ci == 0, ci == len(chunks) - 1)))
                    # emit previous chunk's back half now (its inputs are
                    # ready, so no engine queue blocks)
                    if len(pending) > 1:
                        drain_one(pending)
                    do_finalize()

            def finalize(j):
                pswz = pswz_of.pop(j)
                zs = psm.tile([128, H], f32, tag="zs")
                nc.vector.tensor_tensor(zs[:], acc[j][:, C:CZ],
                                        pswz[:, C:CZ], op=ALU.add)
                zm = psm.tile([128, H], f32, tag="zm")
                nc.scalar.activation(zm[:], zs[:], AF.Copy, bias=1e-30)
                zr = psm.tile([128, H], f32, tag="zr")
                nc.vector.reciprocal(zr[:], zm[:])
                # wvb = wV + bv*z (exact also for isolated nodes: z=0)
                wvb = psm.tile([128, C], f32, tag="wvb")
                nc.vector.tensor_tensor(
                    wvb[:].rearrange("p (h d) -> p h d", d=D),
                    bias_v[:].rearrange("p (h d) -> p h d", d=D),
                    zs[:].unsqueeze(2).broadcast_to([128, H, D]),
                    op=ALU.mult)
                nc.vector.tensor_tensor(wvb[:], pswz[:, 0:C], wvb[:],
                                        op=ALU.add)
                nc.vector.tensor_tensor(wvb[:], acc[j][:, 0:C], wvb[:],
                                        op=ALU.add)
                of = psm.tile([128, C], f32, tag="of")
                nc.vector.tensor_tensor(
                    of[:].rearrange("p (h d) -> p h d", d=D),
                    wvb[:].rearrange("p (h d) -> p h d", d=D),
                    zr[:].unsqueeze(2).broadcast_to([128, H, D]),
                    op=ALU.mult)
                nc.sync.dma_start(out[ts(j, 128), :], of[:])

            # ---- schedule: all ten group-0 items lead, then the rest of
            # phase A, then the pure group-1 stream ----
            for j in range(cfg.NSB):
                process_group(j, 0, kv_t[0][:], last_of_sb=False)
            for cc in range(NCG0, NCALL):
                emit_A(cc)
            for k in range(cfg.NSB):
                process_group(k, 1, kv_t[1][:], last_of_sb=True)
            while pending:
                drain_one(pending)
                do_finalize()
            while pending_fin:
                do_finalize()

    nc.compile()
    return nc


def run(cfg: Cfg, in_maps, trace=False, nc=None):
    if nc is None:
        nc = build(cfg)
    res = run_bass_kernel_spmd(nc, in_maps, core_ids=list(range(cfg.n_cores)),
                               trace=trace)
    full = np.zeros((cfg.NT, cfg.C), dtype=np.float32)
    for i in range(cfg.n_cores):
        o = res.results[i]["out"]
        for j in range(cfg.NSB):
            g_sb = cfg.ASSIGN[i][j]
            full[g_sb * 128:(g_sb + 1) * 128] = o[j * 128:(j + 1) * 128]
    full = full[:cfg.N]
    return full.reshape(cfg.N, cfg.H, cfg.D), res


_PROBLEM_N = 10000
_PROBLEM_IN = 256
_PROBLEM_H = 8
_PROBLEM_D = 32


def kernel(h, Wq, bq, Wk, bk, Wv, bv, src, dst):
    h = np.asarray(h)
    N, IN = h.shape
    C = np.asarray(Wq).shape[1]
    H, D = _PROBLEM_H, _PROBLEM_D
    if C != H * D:
        D = C // H
    src = np.asarray(src)
    dst = np.asarray(dst)
    cfg = make_cfg(N, IN, H, D, src, dst)
    in_maps = prep(cfg, h, Wq, bq, Wk, bk, Wv, bv, src, dst)
    out, _ = run(cfg, in_maps, trace=False)
    return out.astype(np.float32)



# revision 20
# speedup vs baseline: 1.2223x; 1.2223x over previous
"""Self-contained TRN2 Bass kernel for nn_MultiHeadAttentionLayer
(GNN multi-head attention message passing), 8 NeuronCores.

kernel(**inputs) takes the FULL unsharded inputs (h, Wq, bq, Wk, bk, Wv,
bv, src, dst) as numpy arrays and returns the FULL [N, H, D] float32
output. Sharding: edges are partitioned by dst range across the 8 cores
(no collectives needed); each core projects K/V for all nodes into two
src-half tables, gathers K|V rows per edge with dma_gather round-robined
over SWDGE queues 1-3 (async descriptor generation on separate Q7 core
pairs), computes scores/softmax weights on DVE/ACT, and segment-sums
into per-superblock PSUM accumulators via one-hot (fp8) matmuls on the
TensorEngine. The per-chunk back half (exp, V-weighting, scatter) is
software-pipelined one chunk behind the front half so no engine queue
blocks on a cross-engine dependency.
"""

from dataclasses import dataclass, field

import numpy as np
import ml_dtypes

import concourse.bass as bass
import concourse.tile as tile
from concourse import bacc, mybir
from concourse.bass import ts
from concourse.bass_utils import run_bass_kernel_spmd

BF16 = ml_dtypes.bfloat16


def _register_mul_cumsum():
    """Custom DVE op: out = inclusive prefix sum of (Src0*Src1) over the
    free-dim stream. Segmented dot products then fall out of differences
    of segment-end columns."""
    from concourse import dve_ops as DV
    from concourse.dve_spec import Scan, Src0, Src1

    NAME = "MUL_CUMSUM_ANT"
    for op in DV.OPS:
        if op.name == NAME:
            return op
    spec = DV.Spec(
        body=Scan(op=DV.AluOp.ADD, expr=(Src0 * Src1)),
        reference=lambda in0, in1, s0, s1: (in0 * in1).cumsum(axis=-1),
    )
    from concourse.dve_ops import lower, has_src1, DveOpSpec, DveOp
    row = max(DV._SUB_OPCODE_FOR_NAME.values()) + 1
    assert row < 0x20
    shas = {}
    for ver in ("v3", "v4"):
        tmp = DveOpSpec(name=NAME, opcode=row, uops=lower(spec, ver=ver),
                        rd1_en=has_src1(spec))
        shas[ver] = tmp.sha(ver)
    op = DveOp(NAME, spec, subdim=False, uops_sha=shas)
    DV.OPS.append(op)
    DV.CUSTOM_DVE_SPECS[NAME] = spec
    DV._SUB_OPCODE_FOR_NAME[NAME] = row
    return op
F32 = np.float32
AF = mybir.ActivationFunctionType
ALU = mybir.AluOpType

CH_CAP = 12          # max blocks per gather/compute chunk
NGRP = 2             # src groups (table halves)
N_SWDGE_Q = 4        # SWDGE queues allocated (queue 0 unused for gathers:
                     # it is synchronous on the Pool engine; 1-3 are async)
QW = 4               # blocks per Q-gather PSUM piece
WB = 4               # phase-A table-write batch (chunks per DMA)


@dataclass
class Cfg:
    N: int
    IN: int
    H: int
    D: int
    n_cores: int = 8
    NPC: int = 0
    NT: int = 0
    NSB: int = 0
    CHG: list = field(default_factory=list)   # [j][g] blocks
    ASSIGN: list = field(default_factory=list)  # [core][pos] -> global sb

    @property
    def C(self):
        return self.H * self.D

    @property
    def KA(self):
        return self.IN // 128

    @property
    def GBOUND(self):
        return [0, self.NT // 4, self.NT]

    @property
    def NTGS(self):
        gb = self.GBOUND
        return [gb[g + 1] - gb[g] for g in range(NGRP)]

    @property
    def SBLK(self):
        return [sum(row) for row in self.CHG]


def make_cfg(N, IN, H, D, src, dst, n_cores=8):
    cfg = Cfg(N=N, IN=IN, H=H, D=D, n_cores=n_cores)
    cfg.NPC = -(-N // (n_cores * 128)) * 128
    cfg.NT = cfg.NPC * n_cores
    cfg.NSB = cfg.NPC // 128
    src = np.asarray(src)
    dst = np.asarray(dst)
    gsb = dst // 128                       # global super-block of each edge
    nsb_tot = cfg.NSB * n_cores
    grp = (src >= cfg.GBOUND[1]).astype(np.int64)
    counts = np.zeros((nsb_tot, NGRP), dtype=np.int64)
    np.add.at(counts, (np.minimum(gsb, nsb_tot - 1), grp), 1)
    order = np.argsort(-counts.sum(axis=1), kind="stable")
    cfg.ASSIGN = [[int(order[k * n_cores + i]) for k in range(cfg.NSB)]
                  for i in range(n_cores)]
    cfg.CHG = []
    for k in range(cfg.NSB):
        g_sbs = order[k * n_cores:(k + 1) * n_cores]
        cfg.CHG.append([max(1, int(-(-counts[g_sbs, g].max() // 128)))
                        for g in range(NGRP)])
    return cfg


def chunk_plan(cfg):
    """[(j, g, cb, CH, key)] in canonical (j, g) order."""
    plan = []
    key = 0
    for j in range(cfg.NSB):
        for g in range(NGRP):
            CHG = cfg.CHG[j][g]
            cap = CH_CAP if not (g == NGRP - 1 and j == cfg.NSB - 1) \
                else max(4, -(-CHG // 4))
            nparts = -(-CHG // cap)
            base = -(-CHG // nparts)
            b0 = 0
            while b0 < CHG:
                plan.append((j, g, b0, min(base, CHG - b0), key))
                key += 1
                b0 += base
    return plan


def _wrap16(idx, epb):
    base = idx.reshape(epb // 16, 16).T.astype(np.int16)
    return np.tile(base, (8, 1))


def prep(cfg: Cfg, h, Wq, bq, Wk, bk, Wv, bv, src, dst):
    N, IN, H, D, C = cfg.N, cfg.IN, cfg.H, cfg.D, cfg.C
    scale = 1.0 / np.sqrt(np.float32(D))

    hT = np.zeros((IN, cfg.NT), dtype=BF16)
    hT[:, :N] = np.asarray(h).T.astype(BF16)
    wkv = np.concatenate([np.asarray(Wk), np.asarray(Wv)], axis=1).astype(BF16)
    bkv = np.concatenate([np.asarray(bk), np.asarray(bv)])[None, :].astype(BF16)
    wq = (np.asarray(Wq) * scale).astype(BF16)
    bqs = (np.asarray(bq) * scale)[None, :].astype(BF16)

    src = np.asarray(src).astype(np.int64)
    dst = np.asarray(dst).astype(np.int64)

    sum_blk = sum(cfg.SBLK)
    sum_epb = sum_blk * 128

    gsb_of = dst // 128
    grp_of = (src >= cfg.GBOUND[1]).astype(np.int64)
    in_maps = []
    for i in range(cfg.n_cores):
        srcidx = np.zeros(sum_epb, dtype=np.int64)
        ld = np.full((sum_blk, 128), 255, dtype=np.int64)
        off_e = 0
        off_b = 0
        for j in range(cfg.NSB):
            g_sb = cfg.ASSIGN[i][j]
            insb = gsb_of == g_sb
            es, ed, eg = src[insb], dst[insb] - g_sb * 128, grp_of[insb]
            for g in range(NGRP):
                chg = cfg.CHG[j][g]
                gsel = eg == g
                gidx = es[gsel] - cfg.GBOUND[g]
                cnt = gidx.shape[0]
                epb = chg * 128
                assert cnt <= epb, (i, j, g, cnt, epb)
                srcidx[off_e:off_e + cnt] = gidx
                ldj = np.full(epb, 255, dtype=np.int64)
                ldj[:cnt] = ed[gsel]
                ld[off_b:off_b + chg, :] = ldj.reshape(chg, 128)
                off_e += epb
                off_b += chg

        srcw_parts = []
        off = 0
        for j in range(cfg.NSB):
            for g in range(NGRP):
                epb = cfg.CHG[j][g] * 128
                srcw_parts.append(_wrap16(srcidx[off:off + epb], epb))
                off += epb
        srcw = np.concatenate(srcw_parts, axis=1)

        # one-hot dst matrices in fp8 (0/1 exact): halves their DMA vs bf16
        marange = np.arange(128, dtype=np.int64)
        onehot = (ld[:, :, None] == marange[None, None, :])       # [bb, e, m]
        FP8 = mybir.dt.np(mybir.dt.float8e4)
        Sh = np.ascontiguousarray(onehot.transpose(1, 0, 2)).astype(FP8)
        ShT = np.ascontiguousarray(onehot.transpose(2, 0, 1)).astype(FP8)

        cols = np.concatenate(
            [np.arange(cfg.ASSIGN[i][j] * 128, cfg.ASSIGN[i][j] * 128 + 128)
             for j in range(cfg.NSB)])
        in_maps.append({
            "hT": hT,
            "hTq": np.ascontiguousarray(hT[:, cols]),
            "wkv": wkv, "bkv": bkv, "wq": wq, "bq": bqs,
            "srcidx": srcw,
            "Sh": Sh, "ShT": ShT,
        })
    return in_maps


def build(cfg: Cfg):
    MUL_CUMSUM = _register_mul_cumsum()
    N, IN, H, D, C = cfg.N, cfg.IN, cfg.H, cfg.D, cfg.C
    KA = cfg.KA
    C2 = 2 * C
    CZ = C + H
    sum_blk = sum(cfg.SBLK)
    sum_epb = sum_blk * 128
    bf = mybir.dt.bfloat16
    f32 = mybir.dt.float32
    fp8 = mybir.dt.float8e4

    nc = bacc.Bacc("TRN2", target_bir_lowering=False, debug=False,
                   num_swdge_queues=N_SWDGE_Q)
    hT = nc.dram_tensor("hT", [IN, cfg.NT], bf, kind="ExternalInput").ap()
    hTq = nc.dram_tensor("hTq", [IN, cfg.NPC], bf, kind="ExternalInput").ap()
    wkv = nc.dram_tensor("wkv", [IN, C2], bf, kind="ExternalInput").ap()
    bkv = nc.dram_tensor("bkv", [1, C2], bf, kind="ExternalInput").ap()
    wq = nc.dram_tensor("wq", [IN, C], bf, kind="ExternalInput").ap()
    bq = nc.dram_tensor("bq", [1, C], bf, kind="ExternalInput").ap()
    srcidx = nc.dram_tensor("srcidx", [128, sum_epb // 16], mybir.dt.int16,
                            kind="ExternalInput").ap()
    Sh_d = nc.dram_tensor("Sh", [128, sum_blk, 128], fp8,
                          kind="ExternalInput").ap()
    ShT_d = nc.dram_tensor("ShT", [128, sum_blk, 128], fp8,
                           kind="ExternalInput").ap()
    out = nc.dram_tensor("out", [cfg.NPC, C], f32, kind="ExternalOutput").ap()

    with tile.TileContext(nc) as tc:
        with (
            tc.tile_pool(name="dram", bufs=1, space="DRAM") as dramp,
            tc.tile_pool(name="const", bufs=1) as constp,
        ):
            kv_t = [dramp.tile([cfg.NTGS[g], C2], bf, name=f"kv_t{g}")
                    for g in range(NGRP)]

            wkvt = constp.tile([128, KA, C2], bf)
            nc.sync.dma_start(wkvt[:], wkv.rearrange("(a p) c -> p a c", p=128))
            wqt = constp.tile([128, KA, C], bf)
            nc.sync.dma_start(wqt[:], wq.rearrange("(a p) c -> p a c", p=128))
            bkvt = constp.tile([1, C2], bf)
            nc.sync.dma_start(bkvt[:], bkv[:])
            bqt = constp.tile([1, C], bf)
            nc.sync.dma_start(bqt[:], bq[:])
            ones1 = constp.tile([1, 128], bf)
            nc.vector.memset(ones1[:], 1.0)
            srct = constp.tile([128, sum_epb // 16], mybir.dt.int16)
            nc.sync.dma_start(srct[:], srcidx[:])
            qs = constp.tile([128, cfg.NSB, C], bf)
            bias_v = constp.tile([128, C], f32)

            # ---------------- Phase A ----------------
            import contextlib
            pg_ctx = contextlib.ExitStack()
            pg = pg_ctx.enter_context(tc.tile_pool(name="pb_g", bufs=8))
            with (
                tc.tile_pool(name="pa_h", bufs=1) as pah,
                tc.tile_pool(name="pa_ps", bufs=6, space="PSUM") as paps,
                tc.tile_pool(name="pa_bps", bufs=1, space="PSUM") as pabps,
                tc.tile_pool(name="pa_sb", bufs=4) as pasb,
            ):
                bps = pabps.tile([128, C2], f32, tag="bps")
                nc.tensor.matmul(out=bps[:], lhsT=ones1[:], rhs=bkvt[:],
                                 start=True, stop=True)
                nc.vector.tensor_copy(bias_v[:], bps[:, C:C2])
                bpq = pabps.tile([128, C], f32, tag="bpq")
                nc.tensor.matmul(out=bpq[:], lhsT=ones1[:], rhs=bqt[:],
                                 start=True, stop=True)
                bias_q = pasb.tile([128, C], bf, tag="biasq")
                nc.vector.tensor_copy(bias_q[:], bpq[:])

                hts = pah.tile([128, KA, cfg.NT], bf)
                hT_r = hT.rearrange("(a p) n -> p a n", p=128)
                htq = pah.tile([128, KA, cfg.NPC], bf)
                nc.sync.dma_start(htq[:], hTq.rearrange("(a p) n -> p a n", p=128))
                NSPL = 8
                SPL = cfg.NT // NSPL
                for sp in range(NSPL):
                    nc.sync.dma_start(hts[:, :, ts(sp, SPL)], hT_r[:, :, ts(sp, SPL)])

                NCG0 = cfg.NTGS[0] // 128

                def kv_chunk(cc, wbuf):
                    ps = paps.tile([128, C2], f32, tag="psA")
                    for a in range(KA):
                        nc.tensor.matmul(out=ps[:], lhsT=hts[:, a, ts(cc, 128)],
                                         rhs=wkvt[:, a, :], start=(a == 0),
                                         stop=(a == KA - 1))
                    sl = cc % WB
                    # K half raw (bk cancels per-dst), V raw (bv folded into
                    # the finalize as (wV + bv*z)/z)
                    nc.scalar.copy(wbuf[:, sl, :], ps[:])
                    if sl == WB - 1:
                        cc0 = cc - (WB - 1)
                        g = 0 if cc < NCG0 else 1
                        b0 = cc0 - (0 if g == 0 else NCG0)
                        nc.sync.dma_start(
                            kv_t[g].rearrange("(b p) c -> p b c", p=128)
                            [:, b0:b0 + WB, :],
                            wbuf[:])

                for cc in range(NCG0):
                    if cc % WB == 0:
                        wbuf = pasb.tile([128, WB, C2], bf, tag="bufA",
                                         name=f"wbuf{cc}")
                    kv_chunk(cc, wbuf)

                # Q projection between the table halves
                for qc in range(cfg.NSB):
                    psq = paps.tile([128, C], f32, tag="psA", name="psq")
                    for a in range(KA):
                        nc.tensor.matmul(out=psq[:], lhsT=htq[:, a, ts(qc, 128)],
                                         rhs=wqt[:, a, :], start=(a == 0),
                                         stop=(a == KA - 1))
                    nc.vector.tensor_tensor(qs[:, qc, :], psq[:], bias_q[:],
                                            op=ALU.add)

                for cc in range(NCG0, cfg.NT // 128):
                    if cc % WB == 0:
                        wbuf = pasb.tile([128, WB, C2], bf, tag="bufA",
                                         name=f"wbuf{cc}")
                    kv_chunk(cc, wbuf)

            # ---------------- Phase B ----------------
            grp_off = {}
            off_b = 0
            for j in range(cfg.NSB):
                for g in range(NGRP):
                    grp_off[(j, g)] = off_b
                    off_b += cfg.CHG[j][g]

            with (
                tc.tile_pool(name="pb_t", bufs=3) as pt,
                tc.tile_pool(name="pb_c", bufs=3) as pc,
                tc.tile_pool(name="pb_w", bufs=3) as pw,
                tc.tile_pool(name="pb_s", bufs=2) as psm,
                tc.tile_pool(name="pb_ps", bufs=4, space="PSUM") as pps,
                tc.tile_pool(name="pb_qps", bufs=2, space="PSUM") as pqps,
            ):
                pswz_of = {}
                gq_counter = [0]
                plan_all = chunk_plan(cfg)
                pending = []          # software-pipelined back halves
                pending_fin = []      # deferred finalizes

                def back_half(st):
                    (j, sc, kvg, sh, wvz, CH, flags) = st
                    pswz = pswz_of[j]
                    nc.scalar.activation(wvz[:, :, C:CZ], sc[:], AF.Exp)
                    nc.vector.tensor_tensor(
                        wvz[:, :, 0:C].rearrange("p b (h d) -> p b h d", d=D),
                        kvg[:, :, C:C2].rearrange("p b (h d) -> p b h d", d=D),
                        wvz[:, :, C:CZ].unsqueeze(3)
                        .broadcast_to([128, CH, H, D]),
                        op=ALU.mult)
                    first, last = flags
                    for b in range(CH):
                        nc.tensor.matmul(
                            out=pswz[:], lhsT=sh[:, b, :], rhs=wvz[:, b, :],
                            start=(first and b == 0),
                            stop=(last and b == CH - 1))
                    if last:
                        pending_fin.append(j)

                def drain_one(q):
                    if q:
                        back_half(q.pop(0))

                def do_finalize():
                    if pending_fin:
                        finalize(pending_fin.pop(0))

                def process_group(j, g, tbl, last_of_sb):
                    gb = grp_off[(j, g)]
                    if j not in pswz_of:
                        pswz_of[j] = pps.tile([128, CZ], f32, tag="pswz",
                                              name=f"pswz{j}")
                    first_of_sb = (g == 0)
                    chunks = [(cb, CH, key) for (jj, gg, cb, CH, key) in plan_all
                              if jj == j and gg == g]
                    for (ci, (cb, CH, key)) in enumerate(chunks):
                        cbk = gb + cb
                        ce = cbk * 128
                        EPC = CH * 128
                        kvg = pg.tile([128, CH, C2], bf, tag="kvg")
                        nc.gpsimd.dma_gather(
                            kvg[:], tbl, srct[:, ce // 16:(ce + EPC) // 16],
                            EPC, EPC, C2, single_packet=False,
                            queue_num=1 + gq_counter[0] % 3)
                        gq_counter[0] += 1

                        sh = pt.tile([128, CH, 128], fp8, tag="sh")
                        nc.sync.dma_start(sh[:], Sh_d[:, cbk:cbk + CH, :])
                        sht = pt.tile([128, CH, 128], fp8, tag="sht")
                        nc.sync.dma_start(sht[:], ShT_d[:, cbk:cbk + CH, :])

                        # --- Q gather (PE) + PSUM->SBUF copy on ACT ---
                        qg = pc.tile([128, CH, C], bf, tag="qg")
                        for b0q in range(0, CH, QW):
                            bw = min(QW, CH - b0q)
                            qps = pqps.tile([128, QW, C], f32, tag="qps")
                            for b in range(b0q, b0q + bw):
                                nc.tensor.matmul(out=qps[:, b - b0q, :],
                                                 lhsT=sht[:, b, :],
                                                 rhs=qs[:, j, :],
                                                 start=True, stop=True)
                            nc.scalar.copy(qg[:, b0q:b0q + bw, :],
                                           qps[:, 0:bw, :])

                        # --- score: cumsum of K*Qg, then segment-end
                        # differences (one custom DVE op instead of a
                        # multiply + log2(D) tree) ---
                        NSEG = CH * H
                        cum = pc.tile([128, CH * C + 1], f32, tag="cum")
                        nc.vector.memset(cum[:, 0:1], 0.0)
                        nc.vector._custom_dve(
                            MUL_CUMSUM, out=cum[:, 1:1 + CH * C],
                            in0=kvg[:, :, 0:C], in1=qg[:])
                        sc = pc.tile([128, CH, H], f32, tag="sc")
                        nc.vector.tensor_tensor(
                            sc[:].rearrange("p b h -> p (b h)"),
                            cum[:, D:CH * C + 1:D],
                            cum[:, 0:NSEG * D:D],
                            op=ALU.subtract)

                        wvz = pw.tile([128, CH, CZ], bf, tag="wvz")
                        pending.append((j, sc, kvg, sh, wvz, CH,
                                        (first_of_sb and ci == 0,
                                         last_of_sb and ci == len(chunks) - 1)))
                        # emit previous chunk's back half now (its inputs are
                        # ready, so no engine queue blocks)
                        if len(pending) > 1:
                            drain_one(pending)
                        do_finalize()

                def finalize(j):
                    pswz = pswz_of.pop(j)
                    zm = psm.tile([128, H], f32, tag="zm")
                    nc.vector.tensor_scalar(zm[:], pswz[:, C:CZ], 1e-30, None,
                                            op0=ALU.max)
                    zr = psm.tile([128, H], f32, tag="zr")
                    nc.vector.reciprocal(zr[:], zm[:])
                    # wvb = wV + bv*z (exact also for isolated nodes: z=0)
                    wvb = psm.tile([128, C], f32, tag="wvb")
                    nc.vector.tensor_tensor(
                        wvb[:].rearrange("p (h d) -> p h d", d=D),
                        bias_v[:].rearrange("p (h d) -> p h d", d=D),
                        pswz[:, C:CZ].unsqueeze(2).broadcast_to([128, H, D]),
                        op=ALU.mult)
                    nc.vector.tensor_tensor(wvb[:], pswz[:, 0:C], wvb[:],
                                            op=ALU.add)
                    of = psm.tile([128, C], f32, tag="of")
                    nc.vector.tensor_tensor(
                        of[:].rearrange("p (h d) -> p h d", d=D),
                        wvb[:].rearrange("p (h d) -> p h d", d=D),
                        zr[:].unsqueeze(2).broadcast_to([128, H, D]),
                        op=ALU.mult)
                    nc.sync.dma_start(out[ts(j, 128), :], of[:])

                NLEAD = min(3, cfg.NSB)
                for j in range(NLEAD):
                    process_group(j, 0, kv_t[0][:], last_of_sb=False)
                for k in range(cfg.NSB):
                    if k + NLEAD < cfg.NSB:
                        process_group(k + NLEAD, 0, kv_t[0][:],
                                      last_of_sb=False)
                    process_group(k, 1, kv_t[1][:], last_of_sb=True)
                while pending:
                    drain_one(pending)
                    do_finalize()
                while pending_fin:
                    do_finalize()
            pg_ctx.close()

    nc.compile()
    return nc


def run(cfg: Cfg, in_maps, trace=False, nc=None):
    if nc is None:
        nc = build(cfg)
    res = run_bass_kernel_spmd(nc, in_maps, core_ids=list(range(cfg.n_cores)),
                               trace=trace)
    full = np.zeros((cfg.NT, cfg.C), dtype=np.float32)
    for i in range(cfg.n_cores):
        o = res.results[i]["out"]
        for j in range(cfg.NSB):
            g_sb = cfg.ASSIGN[i][j]
            full[g_sb * 128:(g_sb + 1) * 128] = o[j * 128:(j + 1) * 128]
    full = full[:cfg.N]
    return full.reshape(cfg.N, cfg.H, cfg.D), res


_PROBLEM_N = 10000
_PROBLEM_IN = 256
_PROBLEM_H = 8
_PROBLEM_D = 32


def kernel(h, Wq, bq, Wk, bk, Wv, bv, src, dst):
    h = np.asarray(h)
    N, IN = h.shape
    C = np.asarray(Wq).shape[1]
    H, D = _PROBLEM_H, _PROBLEM_D
    if C != H * D:
        D = C // H
    src = np.asarray(src)
    dst = np.asarray(dst)
    cfg = make_cfg(N, IN, H, D, src, dst)
    in_maps = prep(cfg, h, Wq, bq, Wk, bk, Wv, bv, src, dst)
    out, _ = run(cfg, in_maps, trace=False)
    return out.astype(np.float32)

